# revision 18
# baseline (speedup 1.0000x reference)
"""Distributed flash-decoding attention kernel for 8 TRN2 NeuronCores.

B=1024 new tokens attend over a 32768-row KV cache plus the new block
(causal within the block). Sequence-parallel: each core handles 4224 keys
(4096 cache + 128 new) against all 1024 queries, single pass per key
tile t (128 keys):
  scores s = kt_t.T @ qt   -> PSUM f32 [128k, 1024q] (2 MMs of 512, bf16 in)
  e = exp(s)               -> SBUF bf16 (one ACT instr per tile; the
                              3-slot PSUM score ring gives the ACT stream
                              two instructions of pipeline slack)
  pv += va_t.T @ e         -> PSUM f32 [128dv, 1024q] (2 accumulating MMs)
  acc += e                 -> SBUF bf16 (DVE TT at 2x; softmax normalizer)

The causally-masked NEW-BLOCK tile is processed FIRST (ORDER starts at
tile 32) so its extra mask-multiply TT sits at pipeline start, not in
the drain tail. PV matmuls lag scores by one 3-tile group. A single PV
accumulation covers all 33 tiles (no chunk split): the tail is last PV
MMs -> pv evac (DVE+ACT halves) -> l = ones.T@acc into a freed score
PSUM slot -> l evac -> partial DMA split across the SP and ACT queues.
ONE ReduceScatter of the [8 q-blocks x 129 rows x 128 q] partial, then
the epilogue (1/l, PE transpose, scale, out DMA).

Prologue: first DMAs issue on three HWDGE queues in parallel (kt tile
on SP, q halves on ACT and DVE) so the first score matmul starts ~1.3us
earlier than a serial SP-only issue stream.

Engine budget per core: ACT 33 exps ~34us (bottleneck), PE 132 MMs
~28us, DVE ~27us. PSUM: 6 banks score ring + 2 PV. All
dependency-carrying buffers are separate pool tiles (dep tracking is
per-tile; monolithic tiles with slice rotation serialize the pipeline).
"""

import os
import sys

import numpy as np

for _p in ("/opt/trn_rl_repo",):
    if os.path.isdir(_p) and _p not in sys.path:
        sys.path.insert(0, _p)

import ml_dtypes  # noqa: E402
import concourse.bacc as bacc  # noqa: E402
import concourse.mybir as mybir  # noqa: E402
import concourse.tile as tile  # noqa: E402
from concourse.bass_utils import run_bass_kernel_spmd  # noqa: E402

N_CORES = 8
B, S, DK, DV = 1024, 32768, 128, 128
S_SH = S // N_CORES  # 4096 cache rows per core
B_SH = B // N_CORES  # 128 new rows per core
NKEY = S_SH + B_SH  # 4224 keys per core
NT = NKEY // 128  # 33 key tiles
RROW = DV + 1  # 129 rows per q-block in the reduce tensor (dv + l)
F32 = mybir.dt.float32
BF16 = mybir.dt.bfloat16
I32 = mybir.dt.int32

# masked new-block tile (NT-1) runs FIRST; DMA chunks put it in its own
# leading chunk so the first score matmul starts as soon as possible.
# Early chunks are small so the first tiles' data lands before the PE
# in-order queue reaches their Ldweights.
KT_CHUNKS = [(NT - 1, 1), (0, 1), (1, 3), (4, 14), (18, 14)]
VA_CHUNKS = [(NT - 1, 1), (0, 4), (4, 14), (18, 14)]
ORDER = [NT - 1] + list(range(NT - 1))
LAG = 1  # PV trails scores by LAG 3-tile groups


def _declare_io(nc):
    return dict(
        kt=nc.dram_tensor("kt", [128, NKEY], BF16, kind="ExternalInput"),
        qt=nc.dram_tensor("qt", [128, B], BF16, kind="ExternalInput"),
        va=nc.dram_tensor("va", [128, NKEY], BF16, kind="ExternalInput"),
        mask=nc.dram_tensor("mask", [128, B], BF16, kind="ExternalInput"),
        ident=nc.dram_tensor("ident", [128, 128], BF16, kind="ExternalInput"),
        out=nc.dram_tensor("out", [B_SH, DV], F32, kind="ExternalOutput"),
    )


def _emit_body(nc, pools, io, part, stage=6, fillers=True, extras=None):
    """One pass of the compute body; writes the [1032, 128] partial to
    `part`. stage: 1=DMA only, 2=+scores, 3=+exp, 4=+PV, 5=+lacc,
    6=full (l reduce + copies + part DMA)."""
    p_in, p_e, p_acc, p_ep, ps_s, ps_pv = (
        pools["p_in"],
        pools["p_e"],
        pools["p_acc"],
        pools["p_ep"],
        pools["ps_s"],
        pools["ps_pv"],
    )

    # ---- input DMAs. First tile's kt on SP, q halves on the ACT and DVE
    # HWDGE queues (all idle before the exp stream starts) so the first
    # score matmul isn't serialized behind one queue's 650ns-per-issue
    # stream. Everything else stays on SP so the steady-state ACT queue
    # carries only exp instructions.
    # PE warm-up tile: memset first so the filler matmul chain starts
    # as early as possible (see below)
    warm_mm = p_in.tile([128, 256], BF16, name="warm_mm", tag="warm_mm")
    nc.vector.memset(warm_mm[:], 0.0)

    # SP issue order is the prologue critical path: qt0 first (gates the
    # very first MM together with kt32), then kt32 and the small lead-in
    # kt chunks (PE's in-order queue hits their Ldweights right after
    # tile 32's MMs). qt1 + mask ride the otherwise-idle ACT queue.
    def sp_dma(pool_name, tag, dram, f, n):
        t_ = p_in.tile([128, n * 128], BF16, name=pool_name, tag=tag)
        nc.sync.dma_start(t_[:], dram[:, f * 128 : (f + n) * 128])
        return t_

    qt_sbs = []
    qh0 = p_in.tile([128, 512], BF16, name="qt0", tag="qt0")
    nc.sync.dma_start(qh0[:], io["qt"][:, 0:512])
    qt_sbs.append(qh0)
    qh1 = p_in.tile([128, 512], BF16, name="qt1", tag="qt1")
    nc.scalar.dma_start(qh1[:], io["qt"][:, 512:1024])
    qt_sbs.append(qh1)
    mask01 = p_in.tile([128, B], BF16, name="mask01", tag="mask")
    nc.scalar.dma_start(mask01[:], io["mask"][:])

    kt_sbs = []
    va_sbs = []
    kt_sbs.append(sp_dma("kt0", "kt0", io["kt"], *KT_CHUNKS[0]))
    kt_sbs.append(sp_dma("kt1", "kt1", io["kt"], *KT_CHUNKS[1]))
    kt_sbs.append(sp_dma("kt2", "kt2", io["kt"], *KT_CHUNKS[2]))
    va_sbs.append(sp_dma("va0", "va0", io["va"], *VA_CHUNKS[0]))
    kt_sbs.append(sp_dma("kt3", "kt3", io["kt"], *KT_CHUNKS[3]))
    va_sbs.append(sp_dma("va1", "va1", io["va"], *VA_CHUNKS[1]))
    va_sbs.append(sp_dma("va2", "va2", io["va"], *VA_CHUNKS[2]))
    kt_sbs.append(sp_dma("kt4", "kt4", io["kt"], *KT_CHUNKS[4]))
    va_sbs.append(sp_dma("va3", "va3", io["va"], *VA_CHUNKS[3]))
    ident = p_in.tile([128, 128], BF16, name="ident", tag="ident")
    nc.sync.dma_start(ident[:], io["ident"][:])
    pools["ident"] = ident

    def chunk_ap(chunks, sbs, t):
        for (f, n), tile_ in zip(chunks, sbs):
            if f <= t < f + n:
                return tile_[:, (t - f) * 128 : (t - f + 1) * 128]
        raise AssertionError(t)

    if stage < 2:
        return

    # PE p-state warm-up: the cost of a matmul drops 788->427->213ns as
    # the PE accumulates ~3us of busy time. The first input DMAs land at
    # ~4.3us, so run filler matmuls on the memset tile meanwhile; the
    # chain is sized to end just as kt32+qt0 arrive, with the ramp past
    # the 3us full-speed threshold.
    pv_ps = ps_pv.tile([128, B], F32, name="pv_ps", tag="pv")
    # (loop timing builds skip the fillers: inside tc.For_i they would
    # serialize against the previous iteration's pv evacuation)
    for _ in range(14 if fillers else 0):
        nc.tensor.matmul(
            pv_ps[:, 0:256],
            warm_mm[:, 0:128],
            warm_mm[:],
            start=True,
            stop=True,
        )
    accs = []
    e_last = []
    if stage >= 5:
        acc0 = p_acc.tile([128, B], BF16, name="acc0", tag="acc")
        nc.vector.memset(acc0[:], 0.0)
        accs.append(acc0)
    e_tiles = {}

    def emit_scores(t):
        s_t = ps_s.tile([128, B], F32, name="s", tag="s", bufs=3)
        kt_ap = chunk_ap(KT_CHUNKS, kt_sbs, t)
        for h in range(2):
            nc.tensor.matmul(
                s_t[:, h * 512 : (h + 1) * 512],
                kt_ap,
                qt_sbs[h][:],
                start=True,
                stop=True,
            )
        if stage < 3:
            return
        e_t = p_e.tile([128, B], BF16, name="e", tag="e")
        e_tiles[t] = e_t
        nc.scalar.activation(
            e_t[:], s_t[:], mybir.ActivationFunctionType.Exp
        )

    def emit_pv(tr, i):
        e_ap = e_tiles[tr][:]
        if tr == NT - 1:
            em = p_e.tile([128, B], BF16, name="em", tag="em", bufs=1)
            nc.vector.tensor_tensor(
                out=em[:], in0=e_ap, in1=mask01[:], op=mybir.AluOpType.mult
            )
            e_ap = em[:]
        va_ap = chunk_ap(VA_CHUNKS, va_sbs, tr)
        for h in range(2):
            nc.tensor.matmul(
                pv_ps[:, h * 512 : (h + 1) * 512],
                va_ap,
                e_ap[:, h * 512 : (h + 1) * 512],
                start=(i == 0),
                stop=(i == NT - 1),
            )
        if stage >= 5 and i < NT - 1:
            # the final tile skips the DVE acc add: l is computed as
            # ones.T @ acc_penultimate + ones.T @ e_last on PE, so the
            # tail doesn't wait for one more DVE pass
            nxt = p_acc.tile([128, B], BF16, name="accn", tag="acc")
            nc.vector.tensor_tensor(
                out=nxt[:], in0=accs[-1][:], in1=e_ap, op=mybir.AluOpType.add
            )
            accs.append(nxt)
        elif stage >= 5:
            e_last.append(e_ap)

    ones_sb = p_ep.tile([128, 1], BF16, name="ones_sb", tag="ones")
    nc.vector.memset(ones_sb[:], 1.0)

    n_groups = (NT + 2) // 3  # 11
    for g in range(n_groups + LAG):
        for j in range(3):
            i = 3 * g + j
            if i < NT:
                emit_scores(ORDER[i])
        if stage >= 4 and g >= LAG:
            for j in range(3):
                i = 3 * (g - LAG) + j
                if i < NT:
                    emit_pv(ORDER[i], i)
    if stage < 6:
        return

    # ---- tail: pv evac (DVE+ACT halves), l = ones.T@acc into a freed
    # score-ring PSUM slot (no wait on the pv evac), l evac, partial DMA
    # split across the SP and ACT queues.
    pv_sb = p_ep.tile([128, B], BF16, name="pv_sb", tag="pvsb")
    nc.vector.tensor_copy(pv_sb[:, 0:512], pv_ps[:, 0:512])
    nc.scalar.copy(pv_sb[:, 512:1024], pv_ps[:, 512:1024])
    l_ps = ps_s.tile([1, B], F32, name="l_ps", tag="s", bufs=3)
    for h in range(2):
        nc.tensor.matmul(
            l_ps[0:1, h * 512 : (h + 1) * 512],
            ones_sb[:],
            accs[-1][:, h * 512 : (h + 1) * 512],
            start=True,
            stop=False,
        )
    for h in range(2):
        nc.tensor.matmul(
            l_ps[0:1, h * 512 : (h + 1) * 512],
            ones_sb[:],
            e_last[0][:, h * 512 : (h + 1) * 512],
            start=False,
            stop=True,
        )
    l_sb = p_ep.tile([1, B], BF16, name="l_sb", tag="lsb")
    nc.vector.tensor_copy(l_sb[0:1, :], l_ps[0:1, :])
    part3 = part.rearrange("(j r) c -> j r c", r=RROW)
    nc.sync.dma_start(
        part3[0:4, 0:DV, :].rearrange("j r c -> r j c"),
        pv_sb[:, 0:512].rearrange("p (j c) -> p j c", j=4),
    )
    nc.scalar.dma_start(
        part3[4:8, 0:DV, :].rearrange("j r c -> r j c"),
        pv_sb[:, 512:1024].rearrange("p (j c) -> p j c", j=4),
    )
    nc.sync.dma_start(
        part3[:, DV : DV + 1, :].rearrange("j o c -> o j c"),
        l_sb[0:1, :].rearrange("o (j c) -> o j c", j=8),
    )


def _emit_epilogue(nc, pools, io, red):
    p_ep, ps_s, ps_pv = pools["p_ep"], pools["ps_s"], pools["ps_pv"]
    ident = pools["ident"]
    r_dv = p_ep.tile([DV, B_SH], BF16, name="r_dv", tag="r_dv")
    nc.sync.dma_start(r_dv[:], red[0:DV, :])
    r_l = p_ep.tile([1, B_SH], BF16, name="r_l", tag="r_l")
    nc.sync.dma_start(r_l[:], red[DV : DV + 1, :])
    linv = p_ep.tile([1, B_SH], F32, name="linv", tag="linv")
    nc.vector.reciprocal(linv[:], r_l[:])
    one1 = p_ep.tile([1, 1], F32, name="one1", tag="one1")
    nc.vector.memset(one1[:], 1.0)

    t_ps = ps_s.tile([128, B_SH], BF16, name="t_ps", tag="s", bufs=3)
    nc.tensor.transpose(t_ps[:], r_dv[:], ident[:])
    lc_ps = ps_pv.tile([128, 1], F32, name="lc_ps", tag="pv")
    nc.tensor.matmul(lc_ps[:], linv[:], one1[:], start=True, stop=True)
    lc_sb = p_ep.tile([128, 1], F32, name="lc_sb", tag="lc_sb")
    nc.vector.tensor_copy(lc_sb[:], lc_ps[:])
    out_sb = p_ep.tile([128, DV], F32, name="out_sb", tag="out_sb")
    nc.vector.tensor_scalar_mul(out_sb[:], t_ps[:], lc_sb[:])
    nc.sync.dma_start(io["out"][:], out_sb[:])


def build_nc(loop_iters: int | None = None, stage: int = 6):
    """loop_iters=None: real kernel (compute + ReduceScatter + epilogue).
    loop_iters=N: timing variant, compute body in tc.For_i (no
    collective -- collectives can't sit inside control flow)."""
    nc = bacc.Bacc(
        "TRN2", target_bir_lowering=False, debug=False, num_devices=N_CORES
    )
    io = _declare_io(nc)
    with tile.TileContext(nc) as tc:
        with (
            tc.tile_pool(name="p_in", bufs=1) as p_in,
            tc.tile_pool(name="p_e", bufs=9) as p_e,
            tc.tile_pool(name="p_acc", bufs=2) as p_acc,
            tc.tile_pool(name="pmisc", bufs=1) as pmisc,
            tc.tile_pool(name="p_ep", bufs=1) as p_ep,
            tc.tile_pool(name="ps_s", bufs=3, space="PSUM") as ps_s,
            tc.tile_pool(name="ps_pv", bufs=1, space="PSUM") as ps_pv,
            tc.tile_pool(name="pdram", bufs=1, space="DRAM") as pdram,
        ):
            pools = dict(
                p_in=p_in, p_e=p_e, p_acc=p_acc, p_ep=p_ep, ps_s=ps_s,
                ps_pv=ps_pv, tc=tc,
            )
            # ACT table prewarm: tiny exp before any real dependency
            warm = pmisc.tile([128, 1], F32, name="warm", tag="warm")
            nc.vector.memset(warm[:], 0.0)
            warm_o = pmisc.tile([128, 1], BF16, name="warm_o", tag="warm_o")
            nc.scalar.activation(
                warm_o[:], warm[:], mybir.ActivationFunctionType.Exp
            )
            part = pdram.tile([8 * RROW, B_SH], BF16, name="part", tag="pa")
            if loop_iters is None:
                red = pdram.tile([RROW, B_SH], BF16, name="red", tag="ra")
                _emit_body(nc, pools, io, part)
                nc.gpsimd.collective_compute(
                    "ReduceScatter",
                    mybir.AluOpType.add,
                    replica_groups=[list(range(N_CORES))],
                    ins=[part.opt()],
                    outs=[red.opt()],
                )
                _emit_epilogue(nc, pools, io, red)
            elif loop_iters == 0:
                # single body pass, no collective (for TimelineSim)
                _emit_body(nc, pools, io, part, stage=stage)
                out_sb = p_ep.tile([B_SH, DV], F32, name="out_sb1", tag="o0")
                nc.vector.memset(out_sb[:], 0.0)
                nc.sync.dma_start(io["out"][:], out_sb[:])
            else:
                with tc.For_i(0, max(loop_iters, 1), 1):
                    _emit_body(nc, pools, io, part, stage=stage, fillers=False)
                out_sb = p_ep.tile([B_SH, DV], F32, name="out_sb0", tag="o0")
                nc.vector.memset(out_sb[:], 0.0)
                nc.sync.dma_start(io["out"][:], out_sb[:])
    nc.compile()
    return nc


_CACHE: dict = {}


def _get_nc():
    if "nc" not in _CACHE:
        _CACHE["nc"] = build_nc()
    return _CACHE["nc"]


def make_in_maps(q, k, v, K_cache, V_cache):
    q = np.asarray(q, np.float32)
    k = np.asarray(k, np.float32)
    v = np.asarray(v, np.float32)
    K_cache = np.asarray(K_cache, np.float32)
    V_cache = np.asarray(V_cache, np.float32)

    scale = 1.0 / np.sqrt(np.float32(DK))
    qt = np.ascontiguousarray((q * scale).T).astype(ml_dtypes.bfloat16)

    in_maps = []
    for c in range(N_CORES):
        Ksh = np.concatenate(
            [K_cache[c * S_SH : (c + 1) * S_SH], k[c * B_SH : (c + 1) * B_SH]],
            axis=0,
        )  # [4224, 128]
        kt = np.ascontiguousarray(Ksh.T).astype(ml_dtypes.bfloat16)
        Vsh = np.concatenate(
            [V_cache[c * S_SH : (c + 1) * S_SH], v[c * B_SH : (c + 1) * B_SH]],
            axis=0,
        )  # [4224, 128]
        # va[p, t*128 + d] = V[t*128 + p, d]  (PE stationary layout)
        va = np.ascontiguousarray(
            Vsh.reshape(NT, 128, DV).transpose(1, 0, 2).reshape(128, NKEY)
        ).astype(ml_dtypes.bfloat16)
        thr = c * B_SH + np.arange(128, dtype=np.float32)
        mask = (
            np.arange(B, dtype=np.float32)[None, :] >= thr[:, None]
        ).astype(ml_dtypes.bfloat16)
        ident = np.eye(128, dtype=ml_dtypes.bfloat16)
        in_maps.append(
            {"kt": kt, "qt": qt, "va": va, "mask": mask, "ident": ident}
        )
    return in_maps


def kernel(q, k, v, K_cache, V_cache):
    in_maps = make_in_maps(q, k, v, K_cache, V_cache)
    res = run_bass_kernel_spmd(
        _get_nc(), in_maps, core_ids=list(range(N_CORES))
    )
    out = np.concatenate(
        [res.results[c]["out"] for c in range(N_CORES)], axis=0
    )
    return np.ascontiguousarray(out, dtype=np.float32)


# revision 23
# speedup vs baseline: 1.0240x; 1.0240x over previous
"""Distributed flash-decoding attention kernel for 8 TRN2 NeuronCores.

B=1024 new tokens attend over a 32768-row KV cache plus the new block
(causal within the block). Sequence-parallel: each core handles 4224 keys
(4096 cache + 128 new) against all 1024 queries, single pass per key
tile t (128 keys):
  scores s = kt_t.T @ qt   -> PSUM f32 [128k, 1024q] (2 MMs of 512, bf16 in)
  e = exp(s)               -> SBUF bf16 (one ACT instr per tile; the
                              3-slot PSUM score ring gives the ACT stream
                              two instructions of pipeline slack)
  pv += va_t.T @ e         -> PSUM f32 [128dv, 1024q] (2 accumulating MMs)
  acc += e                 -> SBUF bf16 (DVE TT at 2x; softmax normalizer)

The causally-masked NEW-BLOCK tile is processed FIRST (ORDER starts at
tile 32) so its extra mask-multiply TT sits at pipeline start, not in
the drain tail. PV matmuls lag scores by one 3-tile group. A single PV
accumulation covers all 33 tiles (no chunk split): the tail is last PV
MMs -> pv evac (DVE+ACT halves) -> l = ones.T@acc into a freed score
PSUM slot -> l evac -> partial DMA split across the SP and ACT queues.
ONE ReduceScatter of the [8 q-blocks x 129 rows x 128 q] partial
(dependent-RS latency measured ~1-2us on this runtime; collective count,
not payload, is what matters), then the epilogue (1/l, PE transpose,
scale, out DMA).

Prologue: qt0 + small lead-in kt chunks issue first on SP; qt1 + mask
ride the otherwise-idle ACT HWDGE queue. ~14 filler matmuls on a memset
tile keep the PE p-state ramp warm while the first input DMAs land
(one-shot builds only; inside tc.For_i they would serialize against the
previous iteration's PSUM evacuation).

Engine budget per core: ACT 33 exps ~34.3us (bottleneck; exp stream is
gap-free in TimelineSim), PE 132 MMs ~28us, DVE ~23us. PSUM: 6 banks
score ring + 2 PV. All
dependency-carrying buffers are separate pool tiles (dep tracking is
per-tile; monolithic tiles with slice rotation serialize the pipeline).
"""

import os
import sys

import numpy as np

for _p in ("/opt/trn_rl_repo",):
    if os.path.isdir(_p) and _p not in sys.path:
        sys.path.insert(0, _p)

import ml_dtypes  # noqa: E402
import concourse.bacc as bacc  # noqa: E402
import concourse.mybir as mybir  # noqa: E402
import concourse.tile as tile  # noqa: E402
from concourse.bass_utils import run_bass_kernel_spmd  # noqa: E402

N_CORES = 8
B, S, DK, DV = 1024, 32768, 128, 128
S_SH = S // N_CORES  # 4096 cache rows per core
B_SH = B // N_CORES  # 128 new rows per core
NKEY = S_SH + B_SH  # 4224 keys per core
NT = NKEY // 128  # 33 key tiles
RROW = DV + 1  # 129 rows per q-block in the reduce tensor (dv + l)
F32 = mybir.dt.float32
BF16 = mybir.dt.bfloat16
I32 = mybir.dt.int32

# masked new-block tile (NT-1) runs FIRST; DMA chunks put it in its own
# leading chunk so the first score matmul starts as soon as possible.
# Early chunks are small so the first tiles' data lands before the PE
# in-order queue reaches their Ldweights.
KT_CHUNKS = [(NT - 1, 1), (0, 1), (1, 3), (4, 14), (18, 14)]
VA_CHUNKS = [(NT - 1, 1), (0, 4), (4, 14), (18, 14)]
ORDER = [NT - 1] + list(range(NT - 1))
LAG = 1  # PV trails scores by LAG 3-tile groups


def _declare_io(nc):
    return dict(
        kt=nc.dram_tensor("kt", [128, NKEY], BF16, kind="ExternalInput"),
        qt=nc.dram_tensor("qt", [128, B], BF16, kind="ExternalInput"),
        va=nc.dram_tensor("va", [128, NKEY], BF16, kind="ExternalInput"),
        mask=nc.dram_tensor("mask", [128, B], BF16, kind="ExternalInput"),
        ident=nc.dram_tensor("ident", [128, 128], BF16, kind="ExternalInput"),
        out=nc.dram_tensor("out", [B_SH, DV], F32, kind="ExternalOutput"),
    )


def _emit_body(nc, pools, io, part, stage=6, fillers=True, extras=None):
    """One pass of the compute body; writes the [1032, 128] partial to
    `part`. stage: 1=DMA only, 2=+scores, 3=+exp, 4=+PV, 5=+lacc,
    6=full (l reduce + copies + part DMA)."""
    p_in, p_e, p_acc, p_ep, ps_s, ps_pv = (
        pools["p_in"],
        pools["p_e"],
        pools["p_acc"],
        pools["p_ep"],
        pools["ps_s"],
        pools["ps_pv"],
    )

    # ---- input DMAs. First tile's kt on SP, q halves on the ACT and DVE
    # HWDGE queues (all idle before the exp stream starts) so the first
    # score matmul isn't serialized behind one queue's 650ns-per-issue
    # stream. Everything else stays on SP so the steady-state ACT queue
    # carries only exp instructions.
    # PE warm-up tile: memset first so the filler matmul chain starts
    # as early as possible (see below)
    warm_mm = p_in.tile([128, 256], BF16, name="warm_mm", tag="warm_mm")
    nc.vector.memset(warm_mm[:], 0.0)

    # SP issue order is the prologue critical path: qt0 first (gates the
    # very first MM together with kt32), then kt32 and the small lead-in
    # kt chunks (PE's in-order queue hits their Ldweights right after
    # tile 32's MMs). qt1 + mask ride the otherwise-idle ACT queue.
    def sp_dma(pool_name, tag, dram, f, n):
        t_ = p_in.tile([128, n * 128], BF16, name=pool_name, tag=tag)
        nc.sync.dma_start(t_[:], dram[:, f * 128 : (f + n) * 128])
        return t_

    qt_sbs = []
    qh0 = p_in.tile([128, 512], BF16, name="qt0", tag="qt0")
    nc.sync.dma_start(qh0[:], io["qt"][:, 0:512])
    qt_sbs.append(qh0)
    qh1 = p_in.tile([128, 512], BF16, name="qt1", tag="qt1")
    nc.scalar.dma_start(qh1[:], io["qt"][:, 512:1024])
    qt_sbs.append(qh1)
    mask01 = p_in.tile([128, B], BF16, name="mask01", tag="mask")
    nc.scalar.dma_start(mask01[:], io["mask"][:])

    kt_sbs = []
    va_sbs = []
    kt_sbs.append(sp_dma("kt0", "kt0", io["kt"], *KT_CHUNKS[0]))
    kt_sbs.append(sp_dma("kt1", "kt1", io["kt"], *KT_CHUNKS[1]))
    kt_sbs.append(sp_dma("kt2", "kt2", io["kt"], *KT_CHUNKS[2]))
    va_sbs.append(sp_dma("va0", "va0", io["va"], *VA_CHUNKS[0]))
    kt_sbs.append(sp_dma("kt3", "kt3", io["kt"], *KT_CHUNKS[3]))
    va_sbs.append(sp_dma("va1", "va1", io["va"], *VA_CHUNKS[1]))
    va_sbs.append(sp_dma("va2", "va2", io["va"], *VA_CHUNKS[2]))
    kt_sbs.append(sp_dma("kt4", "kt4", io["kt"], *KT_CHUNKS[4]))
    va_sbs.append(sp_dma("va3", "va3", io["va"], *VA_CHUNKS[3]))
    ident = p_in.tile([128, 128], BF16, name="ident", tag="ident")
    nc.sync.dma_start(ident[:], io["ident"][:])
    pools["ident"] = ident

    def chunk_ap(chunks, sbs, t):
        for (f, n), tile_ in zip(chunks, sbs):
            if f <= t < f + n:
                return tile_[:, (t - f) * 128 : (t - f + 1) * 128]
        raise AssertionError(t)

    if stage < 2:
        return

    # PE p-state warm-up: the cost of a matmul drops 788->427->213ns as
    # the PE accumulates ~3us of busy time. The first input DMAs land at
    # ~4.3us, so run filler matmuls on the memset tile meanwhile; the
    # chain is sized to end just as kt32+qt0 arrive, with the ramp past
    # the 3us full-speed threshold.
    pv_ps = ps_pv.tile([128, B], F32, name="pv_ps", tag="pv")
    # (loop timing builds skip the fillers: inside tc.For_i they would
    # serialize against the previous iteration's pv evacuation)
    for _ in range(14 if fillers else 0):
        nc.tensor.matmul(
            pv_ps[:, 0:256],
            warm_mm[:, 0:128],
            warm_mm[:],
            start=True,
            stop=True,
        )
    accs = []
    e_last = []
    if stage >= 5:
        acc0 = p_acc.tile([128, B], BF16, name="acc0", tag="acc")
        nc.vector.memset(acc0[:], 0.0)
        accs.append(acc0)
    e_tiles = {}

    def emit_scores(t):
        s_t = ps_s.tile([128, B], F32, name="s", tag="s", bufs=3)
        kt_ap = chunk_ap(KT_CHUNKS, kt_sbs, t)
        for h in range(2):
            nc.tensor.matmul(
                s_t[:, h * 512 : (h + 1) * 512],
                kt_ap,
                qt_sbs[h][:],
                start=True,
                stop=True,
            )
        if stage < 3:
            return
        e_t = p_e.tile([128, B], BF16, name="e", tag="e")
        e_tiles[t] = e_t
        nc.scalar.activation(
            e_t[:], s_t[:], mybir.ActivationFunctionType.Exp
        )

    def emit_pv(tr, i):
        e_ap = e_tiles[tr][:]
        if tr == NT - 1:
            em = p_e.tile([128, B], BF16, name="em", tag="em", bufs=1)
            nc.vector.tensor_tensor(
                out=em[:], in0=e_ap, in1=mask01[:], op=mybir.AluOpType.mult
            )
            e_ap = em[:]
        va_ap = chunk_ap(VA_CHUNKS, va_sbs, tr)
        for h in range(2):
            nc.tensor.matmul(
                pv_ps[:, h * 512 : (h + 1) * 512],
                va_ap,
                e_ap[:, h * 512 : (h + 1) * 512],
                start=(i == 0),
                stop=(i == NT - 1),
            )
        if stage >= 5 and i < NT - 1:
            # the final tile skips the DVE acc add: l is computed as
            # ones.T @ acc_penultimate + ones.T @ e_last on PE, so the
            # tail doesn't wait for one more DVE pass
            nxt = p_acc.tile([128, B], BF16, name="accn", tag="acc")
            nc.vector.tensor_tensor(
                out=nxt[:], in0=accs[-1][:], in1=e_ap, op=mybir.AluOpType.add
            )
            accs.append(nxt)
        elif stage >= 5:
            e_last.append(e_ap)

    ones_sb = p_ep.tile([128, 1], BF16, name="ones_sb", tag="ones")
    nc.vector.memset(ones_sb[:], 1.0)

    n_groups = (NT + 2) // 3  # 11
    for g in range(n_groups + LAG):
        for j in range(3):
            i = 3 * g + j
            if i < NT:
                emit_scores(ORDER[i])
        if stage >= 4 and g >= LAG:
            for j in range(3):
                i = 3 * (g - LAG) + j
                if i < NT:
                    emit_pv(ORDER[i], i)
    if stage < 6:
        return

    # ---- tail: pv evac (DVE+ACT halves), l = ones.T@acc into a freed
    # score-ring PSUM slot (no wait on the pv evac), l evac, partial DMA
    # split across the SP and ACT queues.
    pv_sb = p_ep.tile([128, B], BF16, name="pv_sb", tag="pvsb")
    nc.vector.tensor_copy(pv_sb[:, 0:512], pv_ps[:, 0:512])
    nc.scalar.copy(pv_sb[:, 512:1024], pv_ps[:, 512:1024])
    l_ps = ps_s.tile([1, B], F32, name="l_ps", tag="s", bufs=3)
    for h in range(2):
        nc.tensor.matmul(
            l_ps[0:1, h * 512 : (h + 1) * 512],
            ones_sb[:],
            accs[-1][:, h * 512 : (h + 1) * 512],
            start=True,
            stop=False,
        )
    for h in range(2):
        nc.tensor.matmul(
            l_ps[0:1, h * 512 : (h + 1) * 512],
            ones_sb[:],
            e_last[0][:, h * 512 : (h + 1) * 512],
            start=False,
            stop=True,
        )
    l_sb = p_ep.tile([1, B], BF16, name="l_sb", tag="lsb")
    nc.vector.tensor_copy(l_sb[0:1, :], l_ps[0:1, :])
    part3 = part.rearrange("(j r) c -> j r c", r=RROW)
    nc.sync.dma_start(
        part3[0:4, 0:DV, :].rearrange("j r c -> r j c"),
        pv_sb[:, 0:512].rearrange("p (j c) -> p j c", j=4),
    )
    nc.sync.dma_start(
        part3[4:8, 0:DV, :].rearrange("j r c -> r j c"),
        pv_sb[:, 512:1024].rearrange("p (j c) -> p j c", j=4),
    )
    nc.sync.dma_start(
        part3[:, DV : DV + 1, :].rearrange("j o c -> o j c"),
        l_sb[0:1, :].rearrange("o (j c) -> o j c", j=8),
    )


def _emit_epilogue(nc, pools, io, red):
    p_ep, ps_s, ps_pv = pools["p_ep"], pools["ps_s"], pools["ps_pv"]
    ident = pools["ident"]
    r_dv = p_ep.tile([DV, B_SH], BF16, name="r_dv", tag="r_dv")
    nc.sync.dma_start(r_dv[:], red[0:DV, :])
    r_l = p_ep.tile([1, B_SH], BF16, name="r_l", tag="r_l")
    nc.sync.dma_start(r_l[:], red[DV : DV + 1, :])
    linv = p_ep.tile([1, B_SH], F32, name="linv", tag="linv")
    nc.vector.reciprocal(linv[:], r_l[:])
    one1 = p_ep.tile([1, 1], F32, name="one1", tag="one1")
    nc.vector.memset(one1[:], 1.0)

    t_ps = ps_s.tile([128, B_SH], BF16, name="t_ps", tag="s", bufs=3)
    nc.tensor.transpose(t_ps[:], r_dv[:], ident[:])
    lc_ps = ps_pv.tile([128, 1], F32, name="lc_ps", tag="pv")
    nc.tensor.matmul(lc_ps[:], linv[:], one1[:], start=True, stop=True)
    lc_sb = p_ep.tile([128, 1], F32, name="lc_sb", tag="lc_sb")
    nc.vector.tensor_copy(lc_sb[:], lc_ps[:])
    out_sb = p_ep.tile([128, DV], F32, name="out_sb", tag="out_sb")
    nc.vector.tensor_scalar_mul(out_sb[:], t_ps[:], lc_sb[:])
    nc.sync.dma_start(io["out"][:], out_sb[:])


def build_nc(loop_iters: int | None = None, stage: int = 6):
    """loop_iters=None: real kernel (compute + ReduceScatter + epilogue).
    loop_iters=N: timing variant, compute body in tc.For_i (no
    collective -- collectives can't sit inside control flow)."""
    nc = bacc.Bacc(
        "TRN2", target_bir_lowering=False, debug=False, num_devices=N_CORES
    )
    io = _declare_io(nc)
    with tile.TileContext(nc) as tc:
        with (
            tc.tile_pool(name="p_in", bufs=1) as p_in,
            tc.tile_pool(name="p_e", bufs=9) as p_e,
            tc.tile_pool(name="p_acc", bufs=2) as p_acc,
            tc.tile_pool(name="pmisc", bufs=1) as pmisc,
            tc.tile_pool(name="p_ep", bufs=1) as p_ep,
            tc.tile_pool(name="ps_s", bufs=3, space="PSUM") as ps_s,
            tc.tile_pool(name="ps_pv", bufs=1, space="PSUM") as ps_pv,
            tc.tile_pool(name="pdram", bufs=1, space="DRAM") as pdram,
        ):
            pools = dict(
                p_in=p_in, p_e=p_e, p_acc=p_acc, p_ep=p_ep, ps_s=ps_s,
                ps_pv=ps_pv, tc=tc,
            )
            # ACT table prewarm: tiny exp before any real dependency
            warm = pmisc.tile([128, 1], F32, name="warm", tag="warm")
            nc.vector.memset(warm[:], 0.0)
            warm_o = pmisc.tile([128, 1], BF16, name="warm_o", tag="warm_o")
            nc.scalar.activation(
                warm_o[:], warm[:], mybir.ActivationFunctionType.Exp
            )
            part = pdram.tile([8 * RROW, B_SH], BF16, name="part", tag="pa")
            if loop_iters is None:
                red = pdram.tile([RROW, B_SH], BF16, name="red", tag="ra")
                _emit_body(nc, pools, io, part)
                nc.gpsimd.collective_compute(
                    "ReduceScatter",
                    mybir.AluOpType.add,
                    replica_groups=[list(range(N_CORES))],
                    ins=[part.opt()],
                    outs=[red.opt()],
                )
                _emit_epilogue(nc, pools, io, red)
            elif loop_iters == 0:
                # single body pass, no collective (for TimelineSim)
                _emit_body(nc, pools, io, part, stage=stage)
                out_sb = p_ep.tile([B_SH, DV], F32, name="out_sb1", tag="o0")
                nc.vector.memset(out_sb[:], 0.0)
                nc.sync.dma_start(io["out"][:], out_sb[:])
            else:
                with tc.For_i(0, max(loop_iters, 1), 1):
                    _emit_body(nc, pools, io, part, stage=stage, fillers=False)
                out_sb = p_ep.tile([B_SH, DV], F32, name="out_sb0", tag="o0")
                nc.vector.memset(out_sb[:], 0.0)
                nc.sync.dma_start(io["out"][:], out_sb[:])
    nc.compile()
    return nc


_CACHE: dict = {}


def _get_nc():
    if "nc" not in _CACHE:
        _CACHE["nc"] = build_nc()
    return _CACHE["nc"]


def make_in_maps(q, k, v, K_cache, V_cache):
    q = np.asarray(q, np.float32)
    k = np.asarray(k, np.float32)
    v = np.asarray(v, np.float32)
    K_cache = np.asarray(K_cache, np.float32)
    V_cache = np.asarray(V_cache, np.float32)

    scale = 1.0 / np.sqrt(np.float32(DK))
    qt = np.ascontiguousarray((q * scale).T).astype(ml_dtypes.bfloat16)

    in_maps = []
    for c in range(N_CORES):
        Ksh = np.concatenate(
            [K_cache[c * S_SH : (c + 1) * S_SH], k[c * B_SH : (c + 1) * B_SH]],
            axis=0,
        )  # [4224, 128]
        kt = np.ascontiguousarray(Ksh.T).astype(ml_dtypes.bfloat16)
        Vsh = np.concatenate(
            [V_cache[c * S_SH : (c + 1) * S_SH], v[c * B_SH : (c + 1) * B_SH]],
            axis=0,
        )  # [4224, 128]
        # va[p, t*128 + d] = V[t*128 + p, d]  (PE stationary layout)
        va = np.ascontiguousarray(
            Vsh.reshape(NT, 128, DV).transpose(1, 0, 2).reshape(128, NKEY)
        ).astype(ml_dtypes.bfloat16)
        thr = c * B_SH + np.arange(128, dtype=np.float32)
        mask = (
            np.arange(B, dtype=np.float32)[None, :] >= thr[:, None]
        ).astype(ml_dtypes.bfloat16)
        ident = np.eye(128, dtype=ml_dtypes.bfloat16)
        in_maps.append(
            {"kt": kt, "qt": qt, "va": va, "mask": mask, "ident": ident}
        )
    return in_maps


def kernel(q, k, v, K_cache, V_cache):
    in_maps = make_in_maps(q, k, v, K_cache, V_cache)
    res = run_bass_kernel_spmd(
        _get_nc(), in_maps, core_ids=list(range(N_CORES))
    )
    out = np.concatenate(
        [res.results[c]["out"] for c in range(N_CORES)], axis=0
    )
    return np.ascontiguousarray(out, dtype=np.float32)
